# revision 12
# baseline (speedup 1.0000x reference)
"""Bass/Tile TRN2 kernel for the additive-attention module.

Math (per batch row b):
    x      = inp[b] @ W + bias                      # [1, H]
    scores = tanh(x + context[b]) @ v               # [S]
    scores = where(mask, -inf, scores)
    aw     = softmax(scores)                        # [S]
    attn   = aw @ context[b]                        # [H]
Returns (attn_applied [B,1,H], attn_weights [B,1,S]).

Sharding: pure data parallel over B across 8 NeuronCores; W/b/v replicated.

Per-core dataflow (single pass over the 64 MiB context shard = memory roofline):
  - context arrives in its natural [s=128, h=1024] tile layout (contiguous
    4 KiB DMA descriptors).
  - score phase per tile: VectorE add (ctx + broadcast(x)), ScalarE tanh,
    fused VectorE tensor_tensor_reduce for the v-weighted free-dim reduction.
  - softmax on the tiny [128,16] score matrix: PE transpose, additive
    -1e4*mask, Exp with fused accum, GPSIMD partition all-reduce, reciprocal.
  - weighted sum on TensorE: normalized weight columns as stationary lhsT
    against the still-resident context tiles, accumulated in PSUM.
"""

from contextlib import ExitStack

import numpy as np

import concourse.bacc as bacc
import concourse.bass as bass
import concourse.bass_isa as bass_isa
import concourse.mybir as mybir
import concourse.tile as tile
from concourse.bass import ts
from concourse.bass_utils import run_bass_kernel_spmd
from concourse.masks import make_identity

N_CORES = 8
B, S, D_IN, H = 64, 2048, 512, 1024
B_LOC = B // N_CORES          # 8 batch rows per core
ST = 128                      # s-tile rows (SBUF partitions)
NT = S // ST                  # 16 s-tiles per batch row
NH = H // 512                 # matmul N<=512 halves
KC = D_IN // 128              # 4 contraction chunks for inp @ W

F32 = mybir.dt.float32
BF16 = mybir.dt.bfloat16
U8 = mybir.dt.uint8

CTX_BUFS = 24                 # context tile pool slots (16 live + prefetch)
MASK_BIG = -10000.0           # additive mask; exp(score - 1e4) == 0.0 in f32


def _bcast_ap(t, row_elems, nrows, part, offset_elems):
    """AP reading the same DRAM row of `row_elems` elements into `part`
    partitions (partition step 0)."""
    return bass.AP(
        tensor=t.tensor if isinstance(t, bass.AP) else t,
        offset=offset_elems,
        ap=[[0, part], [1, row_elems]] if nrows == 1 else [[0, part]],
    )


def _body(ctx, tc, inp_d, ctx_d, mask_d, w_d, b_d, v_d, attn_d, aw_d):
    nc = tc.nc
    consts = ctx.enter_context(tc.tile_pool(name="consts", bufs=1))
    dram = ctx.enter_context(tc.tile_pool(name="dram", bufs=1, space="DRAM"))
    ctx_pool = ctx.enter_context(tc.tile_pool(name="ctxp", bufs=CTX_BUFS))
    work = ctx.enter_context(tc.tile_pool(name="work", bufs=3))
    xbb_pool = ctx.enter_context(tc.tile_pool(name="xbb", bufs=3))
    sc_pool = ctx.enter_context(tc.tile_pool(name="sc", bufs=3))
    psum_attn = ctx.enter_context(tc.tile_pool(name="psA", bufs=2, space="PSUM"))
    psum_small = ctx.enter_context(tc.tile_pool(name="psS", bufs=1, space="PSUM"))

    # ---------------- constants ----------------
    identity = consts.tile([128, 128], F32)
    make_identity(nc, identity)
    ones_1xb = consts.tile([1, B_LOC], F32)
    nc.vector.memset(ones_1xb, 1.0)

    w_sb = consts.tile([128, KC, H], F32)
    nc.sync.dma_start(out=w_sb, in_=w_d[:].rearrange("(c k) h -> k c h", k=128))
    b_row = consts.tile([1, H], F32)
    nc.sync.dma_start(out=b_row, in_=b_d[:].rearrange("(o h) -> o h", o=1))

    # v broadcast across 128 partitions straight from DRAM, then to bf16.
    v_bc = consts.tile([128, H], F32)
    nc.sync.dma_start(out=v_bc, in_=bass.AP(tensor=v_d, offset=0, ap=[[0, 128], [1, H]]))
    v_bc16 = consts.tile([128, H], BF16)
    nc.gpsimd.tensor_copy(v_bc16, v_bc)

    # mask in [j=16 partitions, b, r=128] layout (128-byte DMA runs), then
    # pre-scaled additive form.
    mask_sb = consts.tile([NT, B_LOC, ST], U8)
    nc.sync.dma_start(
        out=mask_sb,
        in_=bass.AP(tensor=mask_d, offset=0, ap=[[ST, NT], [S, B_LOC], [1, ST]]),
    )
    mask_big = consts.tile([NT, B_LOC, ST], F32)
    nc.vector.tensor_scalar(
        mask_big, mask_sb, MASK_BIG, None, op0=mybir.AluOpType.mult
    )

    # ---------------- x = inp @ W + bias ----------------
    inp_sb = consts.tile([B_LOC, D_IN], F32)
    nc.sync.dma_start(out=inp_sb, in_=inp_d[:].rearrange("b o k -> (b o) k"))

    inp_t = consts.tile([128, KC, B_LOC], F32)
    for c in range(KC):
        pt = psum_small.tile([128, B_LOC], F32, tag="pss")
        nc.tensor.transpose(pt, inp_sb[:, ts(c, 128)], identity[:B_LOC, :B_LOC])
        nc.vector.tensor_copy(inp_t[:, c, :], pt)

    xb_sb = consts.tile([B_LOC, H], F32)
    for half in range(NH):
        px = psum_small.tile([B_LOC, 512], F32, tag="pss")
        for c in range(KC):
            nc.tensor.matmul(
                px, inp_t[:, c, :], w_sb[:, c, ts(half, 512)],
                start=(c == 0), stop=False,
            )
        # += bias, broadcast over rows via a K=1 matmul with a ones column.
        nc.tensor.matmul(
            px, ones_1xb, b_row[0:1, ts(half, 512)], start=False, stop=True
        )
        nc.vector.tensor_copy(xb_sb[:, ts(half, 512)], px)

    # Round-trip through DRAM so each batch row can be broadcast-loaded
    # across 128 partitions (only DRAM APs allow partition-step-0 reads).
    # gpsimd (SWDGE) DMAs: engine-side instructions can carry the multiple
    # sync waits these dependent transfers need (SP direct-DMA fits one).
    xb_dram = dram.tile([B_LOC, H], F32)
    nc.gpsimd.dma_start(out=xb_dram, in_=xb_sb)

    # ---------------- main loop over batch rows ----------------
    for bi in range(B_LOC):
        xbb = xbb_pool.tile([128, H], F32)
        nc.gpsimd.dma_start(
            out=xbb,
            in_=bass.AP(tensor=xb_dram.tensor, offset=bi * H, ap=[[0, 128], [1, H]]),
        )

        s_all = sc_pool.tile([128, NT], F32)
        cts = []
        for j in range(NT):
            ct = ctx_pool.tile([128, H], F32, tag="ct")
            nc.sync.dma_start(out=ct, in_=ctx_d[bi, ts(j, 128), :])
            cts.append(ct)

            u = work.tile([128, H], F32, tag="u")
            nc.vector.tensor_add(u, ct, xbb)
            t = work.tile([128, H], BF16, tag="t")
            nc.scalar.activation(t, u, mybir.ActivationFunctionType.Tanh)
            # (t * v) with fused free-dim reduction.  The native
            # TENSOR_TENSOR_REDUCE opcode is not supported by the runtime
            # here; the Anthropic custom-DVE AFFINE_MUL_REDUCE (uop table
            # ships with the NEFF) computes out=(in0*1+0)*in1, accum=sum.
            tt = work.tile([128, H], BF16, tag="tt")
            nc.vector.affine_mul_reduce(
                tt, s_all[:, j : j + 1], t, v_bc16, 1.0, 0.0
            )

        # ---- softmax over the [NT=16, ST=128] score matrix ----
        pst = psum_small.tile([NT, ST], F32, tag="pst")
        nc.tensor.transpose(pst, s_all, identity)
        sm = sc_pool.tile([NT, ST], F32)
        nc.vector.tensor_add(sm, pst, mask_big[:, bi, :])
        p_sb = sc_pool.tile([NT, ST], F32)
        lsum = sc_pool.tile([NT, 1], F32)
        nc.scalar.activation(
            p_sb, sm, mybir.ActivationFunctionType.Exp, accum_out=lsum
        )
        ltot = sc_pool.tile([NT, 1], F32)
        nc.gpsimd.partition_all_reduce(ltot, lsum, channels=NT,
                                       reduce_op=bass_isa.ReduceOp.add)
        inv = sc_pool.tile([NT, 1], F32)
        nc.vector.reciprocal(inv, ltot)
        aw_sb = sc_pool.tile([NT, ST], F32)
        nc.vector.tensor_scalar_mul(aw_sb, p_sb, inv)
        nc.sync.dma_start(out=aw_d[bi, 0, :], in_=aw_sb)

        # normalized weights back to column layout for the weighted sum
        ppt = psum_small.tile([128, NT], F32, tag="ppt")
        nc.tensor.transpose(ppt, aw_sb, identity[:NT, :NT])
        p_t = sc_pool.tile([128, NT], F32)
        nc.vector.tensor_copy(p_t, ppt)

        # ---- attn = aw @ context ----
        pa = [
            psum_attn.tile([1, 512], F32, tag=f"pa{h}", name=f"pa{h}_{bi}")
            for h in range(NH)
        ]
        for j in range(NT):
            for half in range(NH):
                nc.tensor.matmul(
                    pa[half], p_t[:, j : j + 1], cts[j][:, ts(half, 512)],
                    start=(j == 0), stop=(j == NT - 1),
                )
        att_sb = sc_pool.tile([1, H], F32)
        for half in range(NH):
            nc.scalar.copy(att_sb[:, ts(half, 512)], pa[half])
        nc.sync.dma_start(out=attn_d[bi, 0, :], in_=att_sb)


def _build():
    nc = bacc.Bacc("TRN2", target_bir_lowering=False, debug=False)
    inp_d = nc.declare_dram_parameter("inp", [B_LOC, 1, D_IN], F32, isOutput=False)
    ctx_d = nc.declare_dram_parameter("context", [B_LOC, S, H], F32, isOutput=False)
    mask_d = nc.declare_dram_parameter("mask", [B_LOC, 1, S], U8, isOutput=False)
    w_d = nc.declare_dram_parameter("W", [D_IN, H], F32, isOutput=False)
    b_d = nc.declare_dram_parameter("b", [H], F32, isOutput=False)
    v_d = nc.declare_dram_parameter("v", [H, 1], F32, isOutput=False)
    attn_d = nc.declare_dram_parameter("attn_applied", [B_LOC, 1, H], F32, isOutput=True)
    aw_d = nc.declare_dram_parameter("attn_weights", [B_LOC, 1, S], F32, isOutput=True)

    with tile.TileContext(nc) as tc:
        with ExitStack() as ctx:
            _body(ctx, tc, inp_d[:], ctx_d[:], mask_d, w_d, b_d, v_d,
                  attn_d[:], aw_d[:])
    # Legalizes multi-wait instructions (HW allows 1 sync wait per inst).
    nc.compile()
    return nc


_NC_CACHE = {}


def _get_nc():
    if "nc" not in _NC_CACHE:
        _NC_CACHE["nc"] = _build()
    return _NC_CACHE["nc"]


def kernel(inp, hidden, context, mask, W, b, v, **_ignored):
    inp = np.ascontiguousarray(np.asarray(inp, dtype=np.float32))
    context = np.ascontiguousarray(np.asarray(context, dtype=np.float32))
    mask_u8 = np.ascontiguousarray(np.asarray(mask).astype(np.uint8))
    W = np.ascontiguousarray(np.asarray(W, dtype=np.float32))
    b = np.ascontiguousarray(np.asarray(b, dtype=np.float32))
    v = np.ascontiguousarray(np.asarray(v, dtype=np.float32))

    nc = _get_nc()
    in_maps = []
    for i in range(N_CORES):
        sl = slice(i * B_LOC, (i + 1) * B_LOC)
        in_maps.append({
            "inp": inp[sl],
            "context": context[sl],
            "mask": mask_u8[sl],
            "W": W,
            "b": b,
            "v": v,
        })
    res = run_bass_kernel_spmd(nc, in_maps, list(range(N_CORES))).results
    attn = np.concatenate([res[i]["attn_applied"] for i in range(N_CORES)], axis=0)
    aw = np.concatenate([res[i]["attn_weights"] for i in range(N_CORES)], axis=0)
    return attn, aw


# revision 13
# speedup vs baseline: 1.0431x; 1.0431x over previous
"""Bass/Tile TRN2 kernel for the additive-attention module.

Math (per batch row b):
    x      = inp[b] @ W + bias                      # [1, H]
    scores = tanh(x + context[b]) @ v               # [S]
    scores = where(mask, -inf, scores)
    aw     = softmax(scores)                        # [S]
    attn   = aw @ context[b]                        # [H]
Returns (attn_applied [B,1,H], attn_weights [B,1,S]).

Sharding: pure data parallel over B across 8 NeuronCores; W/b/v replicated.

Per-core dataflow (single pass over the 64 MiB context shard = memory roofline):
  - context arrives in its natural [s=128, h=1024] tile layout (contiguous
    4 KiB DMA descriptors).
  - score phase per tile: VectorE add (ctx + broadcast(x)), ScalarE tanh,
    fused VectorE tensor_tensor_reduce for the v-weighted free-dim reduction.
  - softmax on the tiny [128,16] score matrix: PE transpose, additive
    -1e4*mask, Exp with fused accum, GPSIMD partition all-reduce, reciprocal.
  - weighted sum on TensorE: normalized weight columns as stationary lhsT
    against the still-resident context tiles, accumulated in PSUM.
"""

from contextlib import ExitStack

import numpy as np

import concourse.bacc as bacc
import concourse.bass as bass
import concourse.bass_isa as bass_isa
import concourse.mybir as mybir
import concourse.tile as tile
from concourse.bass import ts
from concourse.bass_utils import run_bass_kernel_spmd
from concourse.masks import make_identity

N_CORES = 8
B, S, D_IN, H = 64, 2048, 512, 1024
B_LOC = B // N_CORES          # 8 batch rows per core
ST = 128                      # s-tile rows (SBUF partitions)
NT = S // ST                  # 16 s-tiles per batch row
NH = H // 512                 # matmul N<=512 halves
KC = D_IN // 128              # 4 contraction chunks for inp @ W

F32 = mybir.dt.float32
BF16 = mybir.dt.bfloat16
U8 = mybir.dt.uint8

CTX_BUFS = 10                 # context tiles live only until the add now
U_BUFS = 22                   # bf16 u tiles (16 live per batch + prefetch)
MASK_BIG = -10000.0           # additive mask; exp(score - 1e4) == 0.0 in f32


def _bcast_ap(t, row_elems, nrows, part, offset_elems):
    """AP reading the same DRAM row of `row_elems` elements into `part`
    partitions (partition step 0)."""
    return bass.AP(
        tensor=t.tensor if isinstance(t, bass.AP) else t,
        offset=offset_elems,
        ap=[[0, part], [1, row_elems]] if nrows == 1 else [[0, part]],
    )


def _body(ctx, tc, inp_d, ctx_d, mask_d, w_d, b_d, v_d, attn_d, aw_d):
    nc = tc.nc
    consts = ctx.enter_context(tc.tile_pool(name="consts", bufs=1))
    dram = ctx.enter_context(tc.tile_pool(name="dram", bufs=1, space="DRAM"))
    ctx_pool = ctx.enter_context(tc.tile_pool(name="ctxp", bufs=CTX_BUFS))
    u_pool = ctx.enter_context(tc.tile_pool(name="up", bufs=U_BUFS))
    work = ctx.enter_context(tc.tile_pool(name="work", bufs=3))
    xbb_pool = ctx.enter_context(tc.tile_pool(name="xbb", bufs=3))
    sc_pool = ctx.enter_context(tc.tile_pool(name="sc", bufs=3))
    psum_attn = ctx.enter_context(tc.tile_pool(name="psA", bufs=2, space="PSUM"))
    psum_small = ctx.enter_context(tc.tile_pool(name="psS", bufs=1, space="PSUM"))

    # ---------------- constants ----------------
    identity = consts.tile([128, 128], F32)
    make_identity(nc, identity)
    ones_1xb = consts.tile([1, B_LOC], F32)
    nc.vector.memset(ones_1xb, 1.0)

    w_sb = consts.tile([128, KC, H], F32)
    nc.sync.dma_start(out=w_sb, in_=w_d[:].rearrange("(c k) h -> k c h", k=128))
    b_row = consts.tile([1, H], F32)
    nc.sync.dma_start(out=b_row, in_=b_d[:].rearrange("(o h) -> o h", o=1))

    # v broadcast across 128 partitions straight from DRAM, then to bf16.
    v_bc = consts.tile([128, H], F32)
    nc.sync.dma_start(out=v_bc, in_=bass.AP(tensor=v_d, offset=0, ap=[[0, 128], [1, H]]))
    v_bc16 = consts.tile([128, H], BF16)
    nc.gpsimd.tensor_copy(v_bc16, v_bc)

    # mask in [j=16 partitions, b, r=128] layout (128-byte DMA runs), then
    # pre-scaled additive form.
    mask_sb = consts.tile([NT, B_LOC, ST], U8)
    nc.sync.dma_start(
        out=mask_sb,
        in_=bass.AP(tensor=mask_d, offset=0, ap=[[ST, NT], [S, B_LOC], [1, ST]]),
    )
    mask_big = consts.tile([NT, B_LOC, ST], F32)
    nc.vector.tensor_scalar(
        mask_big, mask_sb, MASK_BIG, None, op0=mybir.AluOpType.mult
    )

    # ---------------- x = inp @ W + bias ----------------
    inp_sb = consts.tile([B_LOC, D_IN], F32)
    nc.sync.dma_start(out=inp_sb, in_=inp_d[:].rearrange("b o k -> (b o) k"))

    inp_t = consts.tile([128, KC, B_LOC], F32)
    for c in range(KC):
        pt = psum_small.tile([128, B_LOC], F32, tag="pss")
        nc.tensor.transpose(pt, inp_sb[:, ts(c, 128)], identity[:B_LOC, :B_LOC])
        nc.vector.tensor_copy(inp_t[:, c, :], pt)

    xb_sb = consts.tile([B_LOC, H], F32)
    for half in range(NH):
        px = psum_small.tile([B_LOC, 512], F32, tag="pss")
        for c in range(KC):
            nc.tensor.matmul(
                px, inp_t[:, c, :], w_sb[:, c, ts(half, 512)],
                start=(c == 0), stop=False,
            )
        # += bias, broadcast over rows via a K=1 matmul with a ones column.
        nc.tensor.matmul(
            px, ones_1xb, b_row[0:1, ts(half, 512)], start=False, stop=True
        )
        nc.vector.tensor_copy(xb_sb[:, ts(half, 512)], px)

    # Round-trip through DRAM so each batch row can be broadcast-loaded
    # across 128 partitions (only DRAM APs allow partition-step-0 reads).
    # gpsimd (SWDGE) DMAs: engine-side instructions can carry the multiple
    # sync waits these dependent transfers need (SP direct-DMA fits one).
    xb_dram = dram.tile([B_LOC, H], F32)
    nc.gpsimd.dma_start(out=xb_dram, in_=xb_sb)

    # ---------------- main loop over batch rows ----------------
    for bi in range(B_LOC):
        xbb = xbb_pool.tile([128, H], F32)
        nc.gpsimd.dma_start(
            out=xbb,
            in_=bass.AP(tensor=xb_dram.tensor, offset=bi * H, ap=[[0, 128], [1, H]]),
        )

        s_all = sc_pool.tile([128, NT], F32)
        us = []
        for j in range(NT):
            ct = ctx_pool.tile([128, H], F32, tag="ct")
            nc.sync.dma_start(out=ct, in_=ctx_d[bi, ts(j, 128), :])

            # u = ctx + x, rounded to bf16.  u16 (not raw ctx) feeds the
            # weighted sum below: attn = sum_s aw_s*(ctx_s) and sum_s aw_s = 1,
            # so attn = (sum_s aw_s*u_s) - x.  bf16 u halves PE matmul cost
            # (fp32 matmuls lower to two passes) and frees ctx tiles early.
            u = u_pool.tile([128, H], BF16, tag="u")
            nc.vector.tensor_add(u, ct, xbb)
            us.append(u)
            t = work.tile([128, H], BF16, tag="t")
            nc.scalar.activation(t, u, mybir.ActivationFunctionType.Tanh)
            # (t * v) with fused free-dim reduction.  The native
            # TENSOR_TENSOR_REDUCE opcode is not supported by the runtime
            # here; the Anthropic custom-DVE AFFINE_MUL_REDUCE (uop table
            # ships with the NEFF) computes out=(in0*1+0)*in1, accum=sum.
            tt = work.tile([128, H], BF16, tag="tt")
            nc.vector.affine_mul_reduce(
                tt, s_all[:, j : j + 1], t, v_bc16, 1.0, 0.0
            )

        # ---- softmax over the [NT=16, ST=128] score matrix ----
        pst = psum_small.tile([NT, ST], F32, tag="pst")
        nc.tensor.transpose(pst, s_all, identity)
        sm = sc_pool.tile([NT, ST], F32)
        nc.vector.tensor_add(sm, pst, mask_big[:, bi, :])
        p_sb = sc_pool.tile([NT, ST], F32)
        lsum = sc_pool.tile([NT, 1], F32)
        nc.scalar.activation(
            p_sb, sm, mybir.ActivationFunctionType.Exp, accum_out=lsum
        )
        ltot = sc_pool.tile([NT, 1], F32)
        nc.gpsimd.partition_all_reduce(ltot, lsum, channels=NT,
                                       reduce_op=bass_isa.ReduceOp.add)
        inv = sc_pool.tile([NT, 1], F32)
        nc.vector.reciprocal(inv, ltot)
        aw_sb = sc_pool.tile([NT, ST], F32)
        nc.vector.tensor_scalar_mul(aw_sb, p_sb, inv)
        nc.sync.dma_start(out=aw_d[bi, 0, :], in_=aw_sb)

        # normalized weights back to column layout for the weighted sum
        ppt = psum_small.tile([128, NT], F32, tag="ppt")
        nc.tensor.transpose(ppt, aw_sb, identity[:NT, :NT])
        p_t = sc_pool.tile([128, NT], BF16)
        nc.vector.tensor_copy(p_t, ppt)

        # ---- attn = aw @ context ----
        pa = [
            psum_attn.tile([1, 512], F32, tag=f"pa{h}", name=f"pa{h}_{bi}")
            for h in range(NH)
        ]
        for j in range(NT):
            for half in range(NH):
                nc.tensor.matmul(
                    pa[half], p_t[:, j : j + 1], us[j][:, ts(half, 512)],
                    start=(j == 0), stop=(j == NT - 1),
                )
        att_sb = sc_pool.tile([1, H], F32)
        for half in range(NH):
            # attn = sum(aw*u) - x   (xbb rows all hold x, use partition 0)
            nc.vector.tensor_sub(
                att_sb[:, ts(half, 512)], pa[half], xbb[0:1, ts(half, 512)]
            )
        nc.sync.dma_start(out=attn_d[bi, 0, :], in_=att_sb)


def _build():
    nc = bacc.Bacc("TRN2", target_bir_lowering=False, debug=False)
    inp_d = nc.declare_dram_parameter("inp", [B_LOC, 1, D_IN], F32, isOutput=False)
    ctx_d = nc.declare_dram_parameter("context", [B_LOC, S, H], F32, isOutput=False)
    mask_d = nc.declare_dram_parameter("mask", [B_LOC, 1, S], U8, isOutput=False)
    w_d = nc.declare_dram_parameter("W", [D_IN, H], F32, isOutput=False)
    b_d = nc.declare_dram_parameter("b", [H], F32, isOutput=False)
    v_d = nc.declare_dram_parameter("v", [H, 1], F32, isOutput=False)
    attn_d = nc.declare_dram_parameter("attn_applied", [B_LOC, 1, H], F32, isOutput=True)
    aw_d = nc.declare_dram_parameter("attn_weights", [B_LOC, 1, S], F32, isOutput=True)

    with tile.TileContext(nc) as tc:
        with ExitStack() as ctx:
            _body(ctx, tc, inp_d[:], ctx_d[:], mask_d, w_d, b_d, v_d,
                  attn_d[:], aw_d[:])
    # Legalizes multi-wait instructions (HW allows 1 sync wait per inst).
    nc.compile()
    return nc


_NC_CACHE = {}


def _get_nc():
    if "nc" not in _NC_CACHE:
        _NC_CACHE["nc"] = _build()
    return _NC_CACHE["nc"]


def kernel(inp, hidden, context, mask, W, b, v, **_ignored):
    inp = np.ascontiguousarray(np.asarray(inp, dtype=np.float32))
    context = np.ascontiguousarray(np.asarray(context, dtype=np.float32))
    mask_u8 = np.ascontiguousarray(np.asarray(mask).astype(np.uint8))
    W = np.ascontiguousarray(np.asarray(W, dtype=np.float32))
    b = np.ascontiguousarray(np.asarray(b, dtype=np.float32))
    v = np.ascontiguousarray(np.asarray(v, dtype=np.float32))

    nc = _get_nc()
    in_maps = []
    for i in range(N_CORES):
        sl = slice(i * B_LOC, (i + 1) * B_LOC)
        in_maps.append({
            "inp": inp[sl],
            "context": context[sl],
            "mask": mask_u8[sl],
            "W": W,
            "b": b,
            "v": v,
        })
    res = run_bass_kernel_spmd(nc, in_maps, list(range(N_CORES))).results
    attn = np.concatenate([res[i]["attn_applied"] for i in range(N_CORES)], axis=0)
    aw = np.concatenate([res[i]["attn_weights"] for i in range(N_CORES)], axis=0)
    return attn, aw
